# revision 14
# baseline (speedup 1.0000x reference)
"""Trainium2 Bass kernel for a single-head attention block (B=8, S=2048, D=512, dk=dv=64).

Sharding: one batch element per NeuronCore (8 cores, data parallel).

Per-core algorithm (batch b), all in "transposed" layouts chosen so that every
matmul contraction runs over the SBUF partition axis:

  host:   qkT[cc] = [q[b].T[64cc:64cc+64]; k[b].T[64cc:64cc+64]]  (interleaved)
          vT = v[b].T                                             [512, 2048]
  proj:   one block-diagonal matmul chain per s-chunk gives qp and kp rows
          packed as [qp; kp] in a single PSUM bank; vp[t,dv] natural layout
  scores: sT[t,s] = sum_d kp[d,t] qp[d,s], t-chunks of 128, row-packed in
          pairs on the PE (tile_position (0,0)/(64,0), K=64 each)
  P       = exp(sT * 1/8 + causal_additive_mask) on ACT (scale fused; no
            max-subtraction: scores are O(5) so fp32 exp is exact-safe and
            matches the reference softmax up to rounding)
  AV:     avT[dv,s] = sum_t vpe[t,dv] P[t,s], vpe = [(vp+bv)*E | E] with
          E[t] = exp(pad[t]); row 64 of avT is the softmax denominator
  out:    out[s,dv] = avT[dv,s] / (avT[64,s] + 1e-10)  (PE transpose + DVE)

Matmul dtype is fp16 (10-bit mantissa): ~5e-4 rel error vs the fp32
reference, 1 cycle/row on the PE, 1-pass weight loads, half-size DMA.
"""

import numpy as np

B, S, D, DK, DV = 8, 2048, 512, 64, 64
NCORES = 8
SC = 512              # s-chunk (attention column) width
NSC = S // SC         # 4
NT = S // 128         # 16 t-chunks

CFG = dict(
    # float16: 1 cyc/row matmuls + fast weight load + half DMA, ~5e-4 rel err
    # float32r: TF32-like ~3e-4 but 2-pass weight loads; float32: exact, 4x slow
    qk_dtype="float16",    # q/k projections + scores matmul precision
    v_dtype="float16",     # v projection, P (attention weights), AV matmul
    qk_rowpack=True,       # pack score matmul pairs into PE row groups
    trace=False,           # collect NTFF profile (set by test.py)
)

_prog = None


def _build_program():
    from contextlib import ExitStack

    import concourse.bass as bass
    import concourse.mybir as mybir
    import concourse.tile as tile
    from concourse import bacc

    f32 = mybir.dt.float32
    qkdt = getattr(mybir.dt, CFG["qk_dtype"])
    vdt = getattr(mybir.dt, CFG["v_dtype"])

    nc = bacc.Bacc(
        trn_type="TRN2",
        target_bir_lowering=False,
        debug=False,
        num_devices=NCORES,
    )

    # [cc, h, p, s']: 64-deep D-chunks of q (rows 0:64) and k (rows 64:128),
    # split into s-halves of 1024 for DMA granularity
    qkT_d = nc.dram_tensor("qkT", [8, 2, 128, 1024], qkdt, kind="ExternalInput").ap()
    # [c, h, p, s']: 128-deep D-chunks of v, s-halves
    vT_d = nc.dram_tensor("vT", [4, 2, 128, 1024], vdt, kind="ExternalInput").ap()
    # wqk[cc] = [[Wq[64cc:+64], 0], [0, Wk[64cc:+64]]] (block-diagonal)
    wqk_d = nc.dram_tensor("wqk", [128, 8 * 128], qkdt, kind="ExternalInput").ap()
    # wv packed: [p, (c, m)] with Wv[128c + p, m] at [p, 64c + m]
    wv_d = nc.dram_tensor("wv", [128, 256], vdt, kind="ExternalInput").ap()
    bias_qk_d = nc.dram_tensor("bias_qk", [128, 1], f32, kind="ExternalInput").ap()
    bvrow_d = nc.dram_tensor("bvrow", [1, DV], f32, kind="ExternalInput").ap()
    padT_d = nc.dram_tensor("padT", [128, NT], f32, kind="ExternalInput").ap()
    out_d = nc.dram_tensor("out", [S, DV], f32, kind="ExternalOutput").ap()

    Exp = mybir.ActivationFunctionType.Exp

    with tile.TileContext(nc) as tc:
        with ExitStack() as ctx:
            const = ctx.enter_context(tc.tile_pool(name="const", bufs=1))
            pp = ctx.enter_context(tc.tile_pool(name="pp", bufs=3))
            sbw = ctx.enter_context(tc.tile_pool(name="sbw", bufs=3))
            ps_qk = ctx.enter_context(tc.tile_pool(name="ps_qk", bufs=2, space="PSUM"))
            ps_pj = ctx.enter_context(tc.tile_pool(name="ps_pj", bufs=2, space="PSUM"))
            ps_pjv = ctx.enter_context(tc.tile_pool(name="ps_pjv", bufs=1, space="PSUM"))
            ps_av = ctx.enter_context(tc.tile_pool(name="ps_av", bufs=1, space="PSUM"))

            # ---- constants; DMA issue spread over sync/scalar/gpsimd queues ----
            wqk = const.tile([128, 8 * 128], qkdt, tag="wqk")
            nc.sync.dma_start(out=wqk[:], in_=wqk_d[:])
            padT = const.tile([128, NT], f32, tag="padT")
            nc.scalar.dma_start(out=padT[:], in_=padT_d[:])
            wv = const.tile([128, 256], vdt, tag="wv")
            nc.gpsimd.dma_start(out=wv[:], in_=wv_d[:])
            bias_qk = const.tile([128, 1], f32, tag="bias_qk")
            nc.gpsimd.dma_start(out=bias_qk[:], in_=bias_qk_d[:])
            # bv broadcast across partitions (bv varies along the free axis of vp)
            bvb = const.tile([128, DV], f32, tag="bvb")
            nc.gpsimd.dma_start(out=bvb[:], in_=bvrow_d.partition_broadcast(128))

            # resident input tiles: 16 qk half-tiles + 8 v half-tiles
            qkt = [
                [
                    const.tile(
                        [128, 1024], qkdt, tag=f"qkt{cc}_{h}", name=f"qkt{cc}_{h}"
                    )
                    for h in range(2)
                ]
                for cc in range(8)
            ]
            vt = [
                [
                    const.tile([128, 1024], vdt, tag=f"vt{c}_{h}", name=f"vt{c}_{h}")
                    for h in range(2)
                ]
                for c in range(4)
            ]
            for h in range(2):
                for cc in range(8):
                    nc.sync.dma_start(out=qkt[cc][h][:], in_=qkT_d[cc, h])
                for c in range(4):
                    nc.scalar.dma_start(out=vt[c][h][:], in_=vT_d[c, h])

            # E[t] = exp(pad[t])
            E = const.tile([128, NT], f32, tag="E")
            nc.scalar.activation(E[:], padT[:], Exp)

            # identity for PE transposes
            ident = const.tile([128, 128], f32, tag="ident")
            nc.gpsimd.memset(ident[:], 0.0)
            nc.gpsimd.affine_select(
                out=ident[:],
                in_=ident[:],
                compare_op=mybir.AluOpType.not_equal,
                fill=1.0,
                base=0,
                pattern=[[-1, 128]],
                channel_multiplier=1,
            )

            # shifted causal mask bank (additive): maskbig[u, x] = 0 if
            # x >= u + 384 (allowed) else -1e30. Slice for diagonal tile r:
            # mask_r[u, w] = maskbig[u, w + 384 - 128*r]
            maskbig = const.tile([128, 896], f32, tag="maskbig")
            nc.gpsimd.memset(maskbig[:], 0.0)
            nc.gpsimd.affine_select(
                out=maskbig[:],
                in_=maskbig[:],
                compare_op=mybir.AluOpType.is_ge,
                fill=-1.0e30,
                base=-384,
                pattern=[[1, 896]],
                channel_multiplier=-1,
            )

            # per-s-chunk projections: qkp = [qp; kp], kqp = [kp; qp] (the swap
            # provides both PE partition placements for row-packed score MMs)
            qkp = [
                const.tile([128, SC], qkdt, tag=f"qkp{i}", name=f"qkp{i}")
                for i in range(NSC)
            ]
            kqp = [
                const.tile([128, SC], qkdt, tag=f"kqp{i}", name=f"kqp{i}")
                for i in range(NSC)
            ]
            # vpe4[sc][:, 65r:65r+65] = [(vp_j + bv) * E_j | E_j], j = 4sc + r
            vpe4 = [
                const.tile(
                    [128, 4 * (DV + 1)], vdt, tag=f"vpe4_{i}", name=f"vpe4_{i}"
                )
                for i in range(NSC)
            ]

            for sc in range(NSC):
                h, qh = sc // 2, sc % 2
                ssl = bass.ds(qh * SC, SC)  # within-half s-slice

                # ---- q/k projections (block-diagonal, one accumulation chain)
                pj = ps_pj.tile([128, SC], f32, tag="pj")
                for cc in range(8):
                    nc.tensor.matmul(
                        pj[:, :],
                        wqk[:, bass.ts(cc, 128)],
                        qkt[cc][h][:, ssl],
                        start=(cc == 0),
                        stop=(cc == 7),
                    )
                nc.vector.tensor_scalar_add(qkp[sc][:], pj[:, :], bias_qk[:])
                nc.vector.tensor_scalar_add(
                    kqp[sc][0:64, :], pj[64:128, :], bias_qk[64:128, :]
                )
                nc.vector.tensor_scalar_add(
                    kqp[sc][64:128, :], pj[0:64, :], bias_qk[0:64, :]
                )

                # ---- v projection (natural layout, one accumulation chain/bank)
                pjv = ps_pjv.tile([128, 4 * DV], f32, tag="pjv")
                for c in range(4):
                    for r in range(4):
                        nc.tensor.matmul(
                            pjv[:, bass.ts(r, DV)],
                            vt[c][h][:, bass.ds(128 * (4 * qh + r), 128)],
                            wv[:, bass.ts(c, 64)],
                            start=(c == 0 and r == 0),
                            stop=(c == 3 and r == 3),
                        )
                # vpe4 = [(vp + bv) * E | E], batched over the 4 t-chunks
                vpev = vpe4[sc].rearrange("p (r c) -> p r c", c=DV + 1)[:, :, 0:DV]
                pjvv = pjv.rearrange("p (r c) -> p r c", c=DV)
                Esl = E[:, bass.ts(sc, 4)]
                nc.vector.tensor_add(
                    vpev,
                    pjvv,
                    bvb.rearrange("p (r c) -> p r c", r=1).broadcast_to([128, 4, DV]),
                )
                nc.vector.tensor_mul(vpev, vpev, Esl.broadcast_to([128, 4, DV]))
                nc.vector.tensor_copy(
                    vpe4[sc].rearrange("p (r c) -> p r c", c=DV + 1)[
                        :, :, DV : DV + 1
                    ],
                    Esl.rearrange("p (r c) -> p r c", c=1),
                )

                # ---- attention column sc ----
                av = ps_av.tile([128, SC], f32, tag="av")
                njt = 4 * sc + 4  # active t-chunks in this column
                for g in range(njt // 2):
                    qk = ps_qk.tile([128, 2 * SC], f32, tag="qk")
                    for r2 in range(2):
                        j = 2 * g + r2
                        jc, jr = j // 4, j % 4
                        if CFG["qk_rowpack"] and r2 == 1:
                            # odd j: kp/qp copies living at partitions 64:128
                            # run on PE row group 1, concurrent with even j
                            nc.tensor.matmul(
                                qk[:, bass.ts(r2, SC)],
                                qkp[jc][64:128, bass.ts(jr, 128)],
                                kqp[sc][64:128, :],
                                start=True,
                                stop=True,
                                tile_position=(64, 0),
                            )
                        else:
                            nc.tensor.matmul(
                                qk[:, bass.ts(r2, SC)],
                                kqp[jc][0:64, bass.ts(jr, 128)],
                                qkp[sc][0:64, :],
                                start=True,
                                stop=True,
                                tile_position=(0, 0) if CFG["qk_rowpack"] else None,
                            )
                    for r2 in range(2):
                        j = 2 * g + r2
                        if j >= 4 * sc:  # diagonal tile: additive causal mask
                            rr = j - 4 * sc
                            w_hi = 128 * (rr + 1)
                            nc.vector.tensor_add(
                                qk[:, r2 * SC : r2 * SC + w_hi],
                                qk[:, r2 * SC : r2 * SC + w_hi],
                                maskbig[:, 384 - 128 * rr : 384 - 128 * rr + w_hi],
                            )
                    P = pp.tile([128, 2 * SC], vdt, tag="P")
                    nc.scalar.activation(P[:], qk[:], Exp, scale=0.125)
                    for r2 in range(2):
                        j = 2 * g + r2
                        nc.tensor.matmul(
                            av[0 : DV + 1, :],
                            vpe4[j // 4][:, bass.ds(65 * (j % 4), DV + 1)],
                            P[:, bass.ts(r2, SC)],
                            start=(j == 0),
                            stop=(j == njt - 1),
                        )

                # ---- column postprocess: transpose avT back + normalize ----
                avsb = sbw.tile([DV + 1, SC], f32, tag="avsb")
                nc.vector.tensor_copy(avsb[:], av[0 : DV + 1, :])
                for m in range(SC // 128):
                    tp = ps_av.tile([128, SC], f32, tag="av")
                    nc.tensor.transpose(
                        tp[:, 0 : DV + 1],
                        avsb[:, bass.ts(m, 128)],
                        ident[0 : DV + 1, 0 : DV + 1],
                    )
                    rcp = sbw.tile([128, 1], f32, tag="rcp")
                    nc.vector.tensor_scalar_add(rcp[:], tp[:, DV : DV + 1], 1e-10)
                    nc.vector.reciprocal(rcp[:], rcp[:])
                    ot = sbw.tile([128, DV], f32, tag="ot")
                    nc.vector.tensor_scalar_mul(ot[:], tp[:, 0:DV], rcp[:])
                    nc.gpsimd.dma_start(
                        out=out_d[bass.ds(sc * SC + m * 128, 128), :], in_=ot[:]
                    )

    nc.compile()
    return nc


def _in_maps(inputs):
    import ml_dtypes

    np_of = {"bfloat16": ml_dtypes.bfloat16, "float16": np.float16}
    qk_np = np_of.get(CFG["qk_dtype"], np.float32)
    v_np = np_of.get(CFG["v_dtype"], np.float32)
    q = np.asarray(inputs["q"], dtype=np.float32)
    k = np.asarray(inputs["k"], dtype=np.float32)
    v = np.asarray(inputs["v"], dtype=np.float32)
    pad = np.asarray(inputs["pad_masks"], dtype=np.float32)
    Wq = np.asarray(inputs["Wq"], dtype=np.float32)
    Wk = np.asarray(inputs["Wk"], dtype=np.float32)
    Wv = np.asarray(inputs["Wv"], dtype=np.float32)
    bq = np.asarray(inputs["bq"], dtype=np.float32)
    bk = np.asarray(inputs["bk"], dtype=np.float32)
    bv = np.asarray(inputs["bv"], dtype=np.float32)

    # block-diagonal q/k weights: wqk[p, 128cc + m] over 64-deep D-chunks
    wqk_p = np.zeros((128, 8 * 128), np.float32)
    for cc in range(8):
        wqk_p[0:64, 128 * cc : 128 * cc + 64] = Wq[64 * cc : 64 * cc + 64, :]
        wqk_p[64:128, 128 * cc + 64 : 128 * cc + 128] = Wk[
            64 * cc : 64 * cc + 64, :
        ]
    wqk_p = wqk_p.astype(qk_np)
    # wv packed [p, (c, m)]
    wv_p = np.ascontiguousarray(
        Wv.reshape(4, 128, 64).transpose(1, 0, 2).reshape(128, 256)
    ).astype(v_np)
    bias_qk = np.ascontiguousarray(np.concatenate([bq, bk]).reshape(128, 1))

    maps = []
    for b in range(B):
        qkcat = np.concatenate(
            [q[b].T.reshape(8, 64, S), k[b].T.reshape(8, 64, S)], axis=1
        )  # [8, 128, 2048]
        maps.append(
            {
                "qkT": np.ascontiguousarray(
                    qkcat.reshape(8, 128, 2, 1024)
                    .transpose(0, 2, 1, 3)
                    .astype(qk_np)
                ),
                "vT": np.ascontiguousarray(
                    v[b].T.reshape(4, 128, 2, 1024)
                    .transpose(0, 2, 1, 3)
                    .astype(v_np)
                ),
                "wqk": wqk_p,
                "wv": wv_p,
                "bias_qk": bias_qk,
                "bvrow": np.ascontiguousarray(bv.reshape(1, DV)),
                "padT": np.ascontiguousarray(pad[b, 0].reshape(NT, 128).T),
            }
        )
    return maps


def kernel(**inputs) -> np.ndarray:
    global _prog
    if _prog is None:
        _prog = _build_program()
    from concourse.bass_utils import run_bass_kernel_spmd

    res = run_bass_kernel_spmd(
        _prog, _in_maps(inputs), core_ids=list(range(NCORES)), trace=CFG["trace"]
    )
    kernel.last_result = res
    return np.stack([res.results[i]["out"] for i in range(NCORES)], axis=0)


# revision 15
# speedup vs baseline: 1.1913x; 1.1913x over previous
"""Trainium2 Bass kernel for a single-head attention block (B=8, S=2048, D=512, dk=dv=64).

Sharding: one batch element per NeuronCore (8 cores, data parallel).

Per-core algorithm (batch b), all in "transposed" layouts chosen so that every
matmul contraction runs over the SBUF partition axis:

  host:   qkT[cc] = [q[b].T[64cc:64cc+64]; k[b].T[64cc:64cc+64]]  (interleaved)
          vT = v[b].T                                             [512, 2048]
  proj:   one block-diagonal matmul chain per s-chunk gives qp and kp rows
          packed as [qp; kp] in a single PSUM bank; vp[t,dv] natural layout
  scores: sT[t,s] = sum_d kp[d,t] qp[d,s], t-chunks of 128, row-packed in
          pairs on the PE (tile_position (0,0)/(64,0), K=64 each)
  P       = exp(sT * 1/8 + causal_additive_mask) on ACT (scale fused; no
            max-subtraction: scores are O(5) so fp32 exp is exact-safe and
            matches the reference softmax up to rounding)
  AV:     avT[dv,s] = sum_t vpe[t,dv] P[t,s], vpe = [(vp+bv)*E | E] with
          E[t] = exp(pad[t]); row 64 of avT is the softmax denominator
  out:    out[s,dv] = avT[dv,s] / (avT[64,s] + 1e-10)  (PE transpose + DVE)

Matmul dtype is fp16 (10-bit mantissa): ~5e-4 rel error vs the fp32
reference, 1 cycle/row on the PE, 1-pass weight loads, half-size DMA.
"""

import numpy as np

B, S, D, DK, DV = 8, 2048, 512, 64, 64
NCORES = 8
SC = 512              # s-chunk (attention column) width
NSC = S // SC         # 4
NT = S // 128         # 16 t-chunks

CFG = dict(
    # float16: 1 cyc/row matmuls + fast weight load + half DMA, ~5e-4 rel err
    # float32r: TF32-like ~3e-4 but 2-pass weight loads; float32: exact, 4x slow
    qk_dtype="float16",    # q/k projections + scores matmul precision
    v_dtype="float16",     # v projection, P (attention weights), AV matmul
    qk_rowpack=True,       # pack score matmul pairs into PE row groups
    trace=False,           # collect NTFF profile (set by test.py)
)

_prog = None


def _build_program():
    from contextlib import ExitStack

    import concourse.bass as bass
    import concourse.mybir as mybir
    import concourse.tile as tile
    from concourse import bacc

    f32 = mybir.dt.float32
    qkdt = getattr(mybir.dt, CFG["qk_dtype"])
    vdt = getattr(mybir.dt, CFG["v_dtype"])

    nc = bacc.Bacc(
        trn_type="TRN2",
        target_bir_lowering=False,
        debug=False,
        num_devices=NCORES,
    )

    # [cc, h, p, s']: 64-deep D-chunks of q (rows 0:64) and k (rows 64:128),
    # split into s-halves of 1024 for DMA granularity
    qkT_d = nc.dram_tensor("qkT", [8, 2, 128, 1024], qkdt, kind="ExternalInput").ap()
    # [c, h, p, s']: 128-deep D-chunks of v, s-halves
    vT_d = nc.dram_tensor("vT", [4, 2, 128, 1024], vdt, kind="ExternalInput").ap()
    # wqk[cc] = [[Wq[64cc:+64], 0], [0, Wk[64cc:+64]]] (block-diagonal)
    wqk_d = nc.dram_tensor("wqk", [128, 8 * 128], qkdt, kind="ExternalInput").ap()
    # wv packed: [p, (c, m)] with Wv[128c + p, m] at [p, 64c + m]
    wv_d = nc.dram_tensor("wv", [128, 256], vdt, kind="ExternalInput").ap()
    bias_qk_d = nc.dram_tensor("bias_qk", [128, 1], f32, kind="ExternalInput").ap()
    bvrow_d = nc.dram_tensor("bvrow", [1, DV], f32, kind="ExternalInput").ap()
    padT_d = nc.dram_tensor("padT", [128, NT], f32, kind="ExternalInput").ap()
    out_d = nc.dram_tensor("out", [S, DV], f32, kind="ExternalOutput").ap()

    Exp = mybir.ActivationFunctionType.Exp

    with tile.TileContext(nc) as tc:
        with ExitStack() as ctx:
            const = ctx.enter_context(tc.tile_pool(name="const", bufs=1))
            pp = ctx.enter_context(tc.tile_pool(name="pp", bufs=3))
            sbw = ctx.enter_context(tc.tile_pool(name="sbw", bufs=3))
            ps_qk = ctx.enter_context(tc.tile_pool(name="ps_qk", bufs=2, space="PSUM"))
            ps_pj = ctx.enter_context(tc.tile_pool(name="ps_pj", bufs=1, space="PSUM"))
            ps_pjv = ctx.enter_context(tc.tile_pool(name="ps_pjv", bufs=1, space="PSUM"))
            ps_av = ctx.enter_context(tc.tile_pool(name="ps_av", bufs=2, space="PSUM"))

            # ---- constants; DMA issue spread over sync/scalar/gpsimd queues ----
            wqk = const.tile([128, 8 * 128], qkdt, tag="wqk")
            nc.sync.dma_start(out=wqk[:], in_=wqk_d[:])
            padT = const.tile([128, NT], f32, tag="padT")
            nc.scalar.dma_start(out=padT[:], in_=padT_d[:])
            wv = const.tile([128, 256], vdt, tag="wv")
            nc.gpsimd.dma_start(out=wv[:], in_=wv_d[:])
            bias_qk = const.tile([128, 1], f32, tag="bias_qk")
            nc.gpsimd.dma_start(out=bias_qk[:], in_=bias_qk_d[:])
            # bv broadcast across partitions (bv varies along the free axis of vp)
            bvb = const.tile([128, DV], f32, tag="bvb")
            nc.gpsimd.dma_start(out=bvb[:], in_=bvrow_d.partition_broadcast(128))

            # resident input tiles: 16 qk half-tiles + 8 v half-tiles
            qkt = [
                [
                    const.tile(
                        [128, 1024], qkdt, tag=f"qkt{cc}_{h}", name=f"qkt{cc}_{h}"
                    )
                    for h in range(2)
                ]
                for cc in range(8)
            ]
            vt = [
                [
                    const.tile([128, 1024], vdt, tag=f"vt{c}_{h}", name=f"vt{c}_{h}")
                    for h in range(2)
                ]
                for c in range(4)
            ]
            for h in range(2):
                for cc in range(8):
                    nc.sync.dma_start(out=qkt[cc][h][:], in_=qkT_d[cc, h])
                for c in range(4):
                    nc.scalar.dma_start(out=vt[c][h][:], in_=vT_d[c, h])

            # PE warmup: dummy matmuls on the first-arriving weight tile keep
            # the PE HAM activity window busy while inputs stream in, so real
            # matmuls start at the full 2.4 GHz clock
            warm = ps_qk.tile([128, 2 * SC], f32, tag="qk")
            for w in range(12):
                nc.tensor.matmul(
                    warm[:, 0:SC],
                    wqk[:, 0:128],
                    wqk[:, 0:SC],
                    start=True,
                    stop=True,
                )

            # E[t] = exp(pad[t])
            E = const.tile([128, NT], f32, tag="E")
            nc.scalar.activation(E[:], padT[:], Exp)

            # identity for PE transposes
            ident = const.tile([128, 128], f32, tag="ident")
            nc.gpsimd.memset(ident[:], 0.0)
            nc.gpsimd.affine_select(
                out=ident[:],
                in_=ident[:],
                compare_op=mybir.AluOpType.not_equal,
                fill=1.0,
                base=0,
                pattern=[[-1, 128]],
                channel_multiplier=1,
            )

            # shifted causal mask bank (additive): maskbig[u, x] = 0 if
            # x >= u + 384 (allowed) else -1e30. Slice for diagonal tile r:
            # mask_r[u, w] = maskbig[u, w + 384 - 128*r]
            maskbig = const.tile([128, 896], f32, tag="maskbig")
            nc.gpsimd.memset(maskbig[:], 0.0)
            nc.gpsimd.affine_select(
                out=maskbig[:],
                in_=maskbig[:],
                compare_op=mybir.AluOpType.is_ge,
                fill=-1.0e30,
                base=-384,
                pattern=[[1, 896]],
                channel_multiplier=-1,
            )

            # per-s-chunk projections: qkp = [qp; kp], kqp = [kp; qp] (the swap
            # provides both PE partition placements for row-packed score MMs)
            qkp = [
                const.tile([128, SC], qkdt, tag=f"qkp{i}", name=f"qkp{i}")
                for i in range(NSC)
            ]
            kqp = [
                const.tile([128, SC], qkdt, tag=f"kqp{i}", name=f"kqp{i}")
                for i in range(NSC)
            ]
            # vpe4[sc][:, 65r:65r+65] = [(vp_j + bv) * E_j | E_j], j = 4sc + r
            vpe4 = [
                const.tile(
                    [128, 4 * (DV + 1)], vdt, tag=f"vpe4_{i}", name=f"vpe4_{i}"
                )
                for i in range(NSC)
            ]

            for sc in range(NSC):
                h, qh = sc // 2, sc % 2
                ssl = bass.ds(qh * SC, SC)  # within-half s-slice

                # ---- q/k projections (block-diagonal, one accumulation chain)
                pj = ps_pj.tile([128, SC], f32, tag="pj")
                for cc in range(8):
                    nc.tensor.matmul(
                        pj[:, :],
                        wqk[:, bass.ts(cc, 128)],
                        qkt[cc][h][:, ssl],
                        start=(cc == 0),
                        stop=(cc == 7),
                    )
                nc.vector.tensor_scalar_add(qkp[sc][:], pj[:, :], bias_qk[:])
                nc.vector.tensor_scalar_add(
                    kqp[sc][0:64, :], pj[64:128, :], bias_qk[64:128, :]
                )
                nc.vector.tensor_scalar_add(
                    kqp[sc][64:128, :], pj[0:64, :], bias_qk[0:64, :]
                )

                # ---- v projection (natural layout, one accumulation chain/bank)
                pjv = ps_pjv.tile([128, 4 * DV], f32, tag="pjv")
                for c in range(4):
                    for r in range(4):
                        nc.tensor.matmul(
                            pjv[:, bass.ts(r, DV)],
                            vt[c][h][:, bass.ds(128 * (4 * qh + r), 128)],
                            wv[:, bass.ts(c, 64)],
                            start=(c == 0 and r == 0),
                            stop=(c == 3 and r == 3),
                        )
                # vpe4 = [(vp + bv) * E | E], batched over the 4 t-chunks
                vpev = vpe4[sc].rearrange("p (r c) -> p r c", c=DV + 1)[:, :, 0:DV]
                pjvv = pjv.rearrange("p (r c) -> p r c", c=DV)
                Esl = E[:, bass.ts(sc, 4)]
                nc.vector.tensor_add(
                    vpev,
                    pjvv,
                    bvb.rearrange("p (r c) -> p r c", r=1).broadcast_to([128, 4, DV]),
                )
                nc.vector.tensor_mul(vpev, vpev, Esl.broadcast_to([128, 4, DV]))
                nc.vector.tensor_copy(
                    vpe4[sc].rearrange("p (r c) -> p r c", c=DV + 1)[
                        :, :, DV : DV + 1
                    ],
                    Esl.rearrange("p (r c) -> p r c", c=1),
                )

                # ---- attention column sc ----
                av = ps_av.tile([128, SC], f32, tag="av")
                njt = 4 * sc + 4  # active t-chunks in this column
                for g in range(njt // 2):
                    qk = ps_qk.tile([128, 2 * SC], f32, tag="qk")
                    for r2 in range(2):
                        j = 2 * g + r2
                        jc, jr = j // 4, j % 4
                        if CFG["qk_rowpack"] and r2 == 1:
                            # odd j: kp/qp copies living at partitions 64:128
                            # run on PE row group 1, concurrent with even j
                            nc.tensor.matmul(
                                qk[:, bass.ts(r2, SC)],
                                qkp[jc][64:128, bass.ts(jr, 128)],
                                kqp[sc][64:128, :],
                                start=True,
                                stop=True,
                                tile_position=(64, 0),
                            )
                        else:
                            nc.tensor.matmul(
                                qk[:, bass.ts(r2, SC)],
                                kqp[jc][0:64, bass.ts(jr, 128)],
                                qkp[sc][0:64, :],
                                start=True,
                                stop=True,
                                tile_position=(0, 0) if CFG["qk_rowpack"] else None,
                            )
                    for r2 in range(2):
                        j = 2 * g + r2
                        if j >= 4 * sc:  # diagonal tile: additive causal mask
                            rr = j - 4 * sc
                            w_hi = 128 * (rr + 1)
                            nc.vector.tensor_add(
                                qk[:, r2 * SC : r2 * SC + w_hi],
                                qk[:, r2 * SC : r2 * SC + w_hi],
                                maskbig[:, 384 - 128 * rr : 384 - 128 * rr + w_hi],
                            )
                    P = pp.tile([128, 2 * SC], vdt, tag="P")
                    nc.scalar.activation(P[:], qk[:], Exp, scale=0.125)
                    for r2 in range(2):
                        j = 2 * g + r2
                        nc.tensor.matmul(
                            av[0 : DV + 1, :],
                            vpe4[j // 4][:, bass.ds(65 * (j % 4), DV + 1)],
                            P[:, bass.ts(r2, SC)],
                            start=(j == 0),
                            stop=(j == njt - 1),
                        )

                # ---- column postprocess: transpose avT back + normalize ----
                avsb = sbw.tile([DV + 1, SC], f32, tag="avsb")
                nc.vector.tensor_copy(avsb[:], av[0 : DV + 1, :])
                tpb = ps_av.tile([128, SC], f32, tag="av")
                for m in range(SC // 128):
                    nc.tensor.transpose(
                        tpb[:, bass.ds(65 * m, DV + 1)],
                        avsb[:, bass.ts(m, 128)],
                        ident[0 : DV + 1, 0 : DV + 1],
                    )
                tpv = tpb[:, 0 : 4 * 65].rearrange("p (m c) -> p m c", c=DV + 1)
                rcp = sbw.tile([128, 4], f32, tag="rcp")
                nc.vector.tensor_scalar_add(
                    rcp.rearrange("p (m c) -> p m c", c=1), tpv[:, :, DV : DV + 1], 1e-10
                )
                nc.vector.reciprocal(rcp[:], rcp[:])
                ot = sbw.tile([128, 4 * DV], f32, tag="ot")
                otv = ot.rearrange("p (m c) -> p m c", c=DV)
                nc.vector.tensor_mul(
                    otv,
                    tpv[:, :, 0:DV],
                    rcp.rearrange("p (m c) -> p m c", c=1).broadcast_to([128, 4, DV]),
                )
                nc.gpsimd.dma_start(
                    out=out_d[bass.ds(sc * SC, SC), :].rearrange(
                        "(m p) v -> p m v", p=128
                    ),
                    in_=otv,
                )

    nc.compile()
    return nc


def _in_maps(inputs):
    import ml_dtypes

    np_of = {"bfloat16": ml_dtypes.bfloat16, "float16": np.float16}
    qk_np = np_of.get(CFG["qk_dtype"], np.float32)
    v_np = np_of.get(CFG["v_dtype"], np.float32)
    q = np.asarray(inputs["q"], dtype=np.float32)
    k = np.asarray(inputs["k"], dtype=np.float32)
    v = np.asarray(inputs["v"], dtype=np.float32)
    pad = np.asarray(inputs["pad_masks"], dtype=np.float32)
    Wq = np.asarray(inputs["Wq"], dtype=np.float32)
    Wk = np.asarray(inputs["Wk"], dtype=np.float32)
    Wv = np.asarray(inputs["Wv"], dtype=np.float32)
    bq = np.asarray(inputs["bq"], dtype=np.float32)
    bk = np.asarray(inputs["bk"], dtype=np.float32)
    bv = np.asarray(inputs["bv"], dtype=np.float32)

    # block-diagonal q/k weights: wqk[p, 128cc + m] over 64-deep D-chunks
    wqk_p = np.zeros((128, 8 * 128), np.float32)
    for cc in range(8):
        wqk_p[0:64, 128 * cc : 128 * cc + 64] = Wq[64 * cc : 64 * cc + 64, :]
        wqk_p[64:128, 128 * cc + 64 : 128 * cc + 128] = Wk[
            64 * cc : 64 * cc + 64, :
        ]
    wqk_p = wqk_p.astype(qk_np)
    # wv packed [p, (c, m)]
    wv_p = np.ascontiguousarray(
        Wv.reshape(4, 128, 64).transpose(1, 0, 2).reshape(128, 256)
    ).astype(v_np)
    bias_qk = np.ascontiguousarray(np.concatenate([bq, bk]).reshape(128, 1))

    maps = []
    for b in range(B):
        qkcat = np.concatenate(
            [q[b].T.reshape(8, 64, S), k[b].T.reshape(8, 64, S)], axis=1
        )  # [8, 128, 2048]
        maps.append(
            {
                "qkT": np.ascontiguousarray(
                    qkcat.reshape(8, 128, 2, 1024)
                    .transpose(0, 2, 1, 3)
                    .astype(qk_np)
                ),
                "vT": np.ascontiguousarray(
                    v[b].T.reshape(4, 128, 2, 1024)
                    .transpose(0, 2, 1, 3)
                    .astype(v_np)
                ),
                "wqk": wqk_p,
                "wv": wv_p,
                "bias_qk": bias_qk,
                "bvrow": np.ascontiguousarray(bv.reshape(1, DV)),
                "padT": np.ascontiguousarray(pad[b, 0].reshape(NT, 128).T),
            }
        )
    return maps


def kernel(**inputs) -> np.ndarray:
    global _prog
    if _prog is None:
        _prog = _build_program()
    from concourse.bass_utils import run_bass_kernel_spmd

    res = run_bass_kernel_spmd(
        _prog, _in_maps(inputs), core_ids=list(range(NCORES)), trace=CFG["trace"]
    )
    kernel.last_result = res
    return np.stack([res.results[i]["out"] for i in range(NCORES)], axis=0)


# revision 16
# speedup vs baseline: 1.2304x; 1.0328x over previous
"""Trainium2 Bass kernel for a single-head attention block (B=8, S=2048, D=512, dk=dv=64).

Sharding: one batch element per NeuronCore (8 cores, data parallel).

Per-core algorithm (batch b), all in "transposed" layouts chosen so that every
matmul contraction runs over the SBUF partition axis:

  host:   qkT[cc] = [q[b].T[64cc:64cc+64]; k[b].T[64cc:64cc+64]]  (interleaved)
          vT = v[b].T                                             [512, 2048]
  proj:   one block-diagonal matmul chain per s-chunk gives qp and kp rows
          packed as [qp; kp] in a single PSUM bank; vp[t,dv] natural layout
  scores: sT[t,s] = sum_d kp[d,t] qp[d,s], t-chunks of 128, row-packed in
          pairs on the PE (tile_position (0,0)/(64,0), K=64 each)
  P       = exp(sT * 1/8 + causal_additive_mask) on ACT (scale fused; no
            max-subtraction: scores are O(5) so fp32 exp is exact-safe and
            matches the reference softmax up to rounding)
  AV:     avT[dv,s] = sum_t vpe[t,dv] P[t,s], vpe = [(vp+bv)*E | E] with
          E[t] = exp(pad[t]); row 64 of avT is the softmax denominator
  out:    out[s,dv] = avT[dv,s] / (avT[64,s] + 1e-10)  (PE transpose + DVE)

Matmul dtype is fp16 (10-bit mantissa): ~5e-4 rel error vs the fp32
reference, 1 cycle/row on the PE, 1-pass weight loads, half-size DMA.
"""

import numpy as np

B, S, D, DK, DV = 8, 2048, 512, 64, 64
NCORES = 8
SC = 512              # s-chunk (attention column) width
NSC = S // SC         # 4
NT = S // 128         # 16 t-chunks

CFG = dict(
    # float16: 1 cyc/row matmuls + fast weight load + half DMA, ~5e-4 rel err
    # float32r: TF32-like ~3e-4 but 2-pass weight loads; float32: exact, 4x slow
    qk_dtype="float16",    # q/k projections + scores matmul precision
    v_dtype="float16",     # v projection, P (attention weights), AV matmul
    qk_rowpack=True,       # pack score matmul pairs into PE row groups
    trace=False,           # collect NTFF profile (set by test.py)
)

_prog = None


def _build_program():
    from contextlib import ExitStack

    import concourse.bass as bass
    import concourse.mybir as mybir
    import concourse.tile as tile
    from concourse import bacc

    f32 = mybir.dt.float32
    qkdt = getattr(mybir.dt, CFG["qk_dtype"])
    vdt = getattr(mybir.dt, CFG["v_dtype"])

    nc = bacc.Bacc(
        trn_type="TRN2",
        target_bir_lowering=False,
        debug=False,
        num_devices=NCORES,
    )

    # [cc, h, p, s']: 64-deep D-chunks of q (rows 0:64) and k (rows 64:128),
    # split into s-halves of 1024 for DMA granularity
    qkT_d = nc.dram_tensor("qkT", [8, 2, 128, 1024], qkdt, kind="ExternalInput").ap()
    # [c, h, p, s']: 128-deep D-chunks of v, s-halves
    vT_d = nc.dram_tensor("vT", [4, 2, 128, 1024], vdt, kind="ExternalInput").ap()
    # wqk[cc] = [[Wq[64cc:+64], 0], [0, Wk[64cc:+64]]] (block-diagonal)
    wqk_d = nc.dram_tensor("wqk", [128, 8 * 128], qkdt, kind="ExternalInput").ap()
    # wv packed: [p, (c, m)] with Wv[128c + p, m] at [p, 64c + m]
    wv_d = nc.dram_tensor("wv", [128, 256], vdt, kind="ExternalInput").ap()
    bias_qk_d = nc.dram_tensor("bias_qk", [128, 1], f32, kind="ExternalInput").ap()
    bvrow_d = nc.dram_tensor("bvrow", [1, DV], f32, kind="ExternalInput").ap()
    padT_d = nc.dram_tensor("padT", [128, NT], f32, kind="ExternalInput").ap()
    out_d = nc.dram_tensor("out", [S, DV], f32, kind="ExternalOutput").ap()

    Exp = mybir.ActivationFunctionType.Exp

    with tile.TileContext(nc) as tc:
        with ExitStack() as ctx:
            const = ctx.enter_context(tc.tile_pool(name="const", bufs=1))
            pp = ctx.enter_context(tc.tile_pool(name="pp", bufs=3))
            sbw = ctx.enter_context(tc.tile_pool(name="sbw", bufs=3))
            ps_qk = ctx.enter_context(tc.tile_pool(name="ps_qk", bufs=2, space="PSUM"))
            ps_pj = ctx.enter_context(tc.tile_pool(name="ps_pj", bufs=1, space="PSUM"))
            ps_pjv = ctx.enter_context(tc.tile_pool(name="ps_pjv", bufs=1, space="PSUM"))
            ps_av = ctx.enter_context(tc.tile_pool(name="ps_av", bufs=2, space="PSUM"))

            # ---- constants; DMA issue spread over sync/scalar/gpsimd queues ----
            wqk = const.tile([128, 8 * 128], qkdt, tag="wqk")
            nc.sync.dma_start(out=wqk[:], in_=wqk_d[:])
            padT = const.tile([128, NT], f32, tag="padT")
            nc.scalar.dma_start(out=padT[:], in_=padT_d[:])
            wv = const.tile([128, 256], vdt, tag="wv")
            nc.gpsimd.dma_start(out=wv[:], in_=wv_d[:])
            bias_qk = const.tile([128, 1], f32, tag="bias_qk")
            nc.gpsimd.dma_start(out=bias_qk[:], in_=bias_qk_d[:])
            # bv broadcast across partitions (bv varies along the free axis of vp)
            bvb = const.tile([128, DV], f32, tag="bvb")
            nc.gpsimd.dma_start(out=bvb[:], in_=bvrow_d.partition_broadcast(128))

            # resident input tiles: 16 qk half-tiles + 8 v half-tiles
            qkt = [
                [
                    const.tile(
                        [128, 1024], qkdt, tag=f"qkt{cc}_{h}", name=f"qkt{cc}_{h}"
                    )
                    for h in range(2)
                ]
                for cc in range(8)
            ]
            vt = [
                [
                    const.tile([128, 1024], vdt, tag=f"vt{c}_{h}", name=f"vt{c}_{h}")
                    for h in range(2)
                ]
                for c in range(4)
            ]
            for h in range(2):
                for cc in range(8):
                    nc.sync.dma_start(out=qkt[cc][h][:], in_=qkT_d[cc, h])
                for c in range(4):
                    nc.scalar.dma_start(out=vt[c][h][:], in_=vT_d[c, h])

            # PE warmup: dummy matmuls on the first-arriving weight tile keep
            # the PE HAM activity window busy while inputs stream in / between
            # dependency stalls, so real matmuls run at the full 2.4 GHz clock
            warm = ps_qk.tile([128, 2 * SC], f32, tag="qk")

            def warm_mm(n):
                for _ in range(n):
                    nc.tensor.matmul(
                        warm[:, 0:SC],
                        wqk[:, 0:128],
                        wqk[:, 0:SC],
                        start=True,
                        stop=True,
                    )

            warm_mm(6)

            # E[t] = exp(pad[t])
            E = const.tile([128, NT], f32, tag="E")
            nc.scalar.activation(E[:], padT[:], Exp)

            # identity for PE transposes
            ident = const.tile([128, 128], f32, tag="ident")
            nc.gpsimd.memset(ident[:], 0.0)
            nc.gpsimd.affine_select(
                out=ident[:],
                in_=ident[:],
                compare_op=mybir.AluOpType.not_equal,
                fill=1.0,
                base=0,
                pattern=[[-1, 128]],
                channel_multiplier=1,
            )

            # shifted causal mask bank (additive): maskbig[u, x] = 0 if
            # x >= u + 384 (allowed) else -1e30. Slice for diagonal tile r:
            # mask_r[u, w] = maskbig[u, w + 384 - 128*r]
            maskbig = const.tile([128, 896], f32, tag="maskbig")
            nc.gpsimd.memset(maskbig[:], 0.0)
            nc.gpsimd.affine_select(
                out=maskbig[:],
                in_=maskbig[:],
                compare_op=mybir.AluOpType.is_ge,
                fill=-1.0e30,
                base=-384,
                pattern=[[1, 896]],
                channel_multiplier=-1,
            )

            # per-s-chunk projections: qkp = [qp; kp], kqp = [kp; qp] (the swap
            # provides both PE partition placements for row-packed score MMs)
            qkp = [
                const.tile([128, SC], qkdt, tag=f"qkp{i}", name=f"qkp{i}")
                for i in range(NSC)
            ]
            kqp = [
                const.tile([128, SC], qkdt, tag=f"kqp{i}", name=f"kqp{i}")
                for i in range(NSC)
            ]
            # vpe4[sc][:, 65r:65r+65] = [(vp_j + bv) * E_j | E_j], j = 4sc + r
            vpe4 = [
                const.tile(
                    [128, 4 * (DV + 1)], vdt, tag=f"vpe4_{i}", name=f"vpe4_{i}"
                )
                for i in range(NSC)
            ]

            for sc in range(NSC):
                h, qh = sc // 2, sc % 2
                ssl = bass.ds(qh * SC, SC)  # within-half s-slice

                # ---- q/k projections (block-diagonal, one accumulation chain)
                pj = ps_pj.tile([128, SC], f32, tag="pj")
                if sc > 0:
                    warm_mm(2)
                for cc in range(8):
                    if sc == 0:
                        warm_mm(2)
                    nc.tensor.matmul(
                        pj[:, :],
                        wqk[:, bass.ts(cc, 128)],
                        qkt[cc][h][:, ssl],
                        start=(cc == 0),
                        stop=(cc == 7),
                    )
                nc.vector.tensor_scalar_add(qkp[sc][:], pj[:, :], bias_qk[:])
                nc.vector.tensor_scalar_add(
                    kqp[sc][0:64, :], pj[64:128, :], bias_qk[64:128, :]
                )
                nc.vector.tensor_scalar_add(
                    kqp[sc][64:128, :], pj[0:64, :], bias_qk[0:64, :]
                )

                # ---- v projection (natural layout, one accumulation chain/bank)
                pjv = ps_pjv.tile([128, 4 * DV], f32, tag="pjv")
                for c in range(4):
                    for r in range(4):
                        nc.tensor.matmul(
                            pjv[:, bass.ts(r, DV)],
                            vt[c][h][:, bass.ds(128 * (4 * qh + r), 128)],
                            wv[:, bass.ts(c, 64)],
                            start=(c == 0 and r == 0),
                            stop=(c == 3 and r == 3),
                        )
                # vpe4 = [(vp + bv) * E | E], batched over the 4 t-chunks
                vpev = vpe4[sc].rearrange("p (r c) -> p r c", c=DV + 1)[:, :, 0:DV]
                pjvv = pjv.rearrange("p (r c) -> p r c", c=DV)
                Esl = E[:, bass.ts(sc, 4)]
                nc.vector.tensor_add(
                    vpev,
                    pjvv,
                    bvb.rearrange("p (r c) -> p r c", r=1).broadcast_to([128, 4, DV]),
                )
                nc.vector.tensor_mul(vpev, vpev, Esl.broadcast_to([128, 4, DV]))
                nc.vector.tensor_copy(
                    vpe4[sc].rearrange("p (r c) -> p r c", c=DV + 1)[
                        :, :, DV : DV + 1
                    ],
                    Esl.rearrange("p (r c) -> p r c", c=1),
                )

                # ---- attention column sc ----
                av = ps_av.tile([128, SC], f32, tag="av")
                njt = 4 * sc + 4  # active t-chunks in this column
                for g in range(njt // 2):
                    qk = ps_qk.tile([128, 2 * SC], f32, tag="qk")
                    for r2 in range(2):
                        j = 2 * g + r2
                        jc, jr = j // 4, j % 4
                        if CFG["qk_rowpack"] and r2 == 1:
                            # odd j: kp/qp copies living at partitions 64:128
                            # run on PE row group 1, concurrent with even j
                            nc.tensor.matmul(
                                qk[:, bass.ts(r2, SC)],
                                qkp[jc][64:128, bass.ts(jr, 128)],
                                kqp[sc][64:128, :],
                                start=True,
                                stop=True,
                                tile_position=(64, 0),
                            )
                        else:
                            nc.tensor.matmul(
                                qk[:, bass.ts(r2, SC)],
                                kqp[jc][0:64, bass.ts(jr, 128)],
                                qkp[sc][0:64, :],
                                start=True,
                                stop=True,
                                tile_position=(0, 0) if CFG["qk_rowpack"] else None,
                            )
                    for r2 in range(2):
                        j = 2 * g + r2
                        if j >= 4 * sc:  # diagonal tile: additive causal mask
                            rr = j - 4 * sc
                            w_hi = 128 * (rr + 1)
                            nc.vector.tensor_add(
                                qk[:, r2 * SC : r2 * SC + w_hi],
                                qk[:, r2 * SC : r2 * SC + w_hi],
                                maskbig[:, 384 - 128 * rr : 384 - 128 * rr + w_hi],
                            )
                    P = pp.tile([128, 2 * SC], vdt, tag="P")
                    nc.scalar.activation(P[:], qk[:], Exp, scale=0.125)
                    for r2 in range(2):
                        j = 2 * g + r2
                        nc.tensor.matmul(
                            av[0 : DV + 1, :],
                            vpe4[j // 4][:, bass.ds(65 * (j % 4), DV + 1)],
                            P[:, bass.ts(r2, SC)],
                            start=(j == 0),
                            stop=(j == njt - 1),
                        )

                # ---- column postprocess: transpose avT back + normalize ----
                avsb = sbw.tile([DV + 1, SC], f32, tag="avsb")
                nc.vector.tensor_copy(avsb[:], av[0 : DV + 1, :])
                tpb = ps_av.tile([128, SC], f32, tag="av")
                for m in range(SC // 128):
                    nc.tensor.transpose(
                        tpb[:, bass.ds(65 * m, DV + 1)],
                        avsb[:, bass.ts(m, 128)],
                        ident[0 : DV + 1, 0 : DV + 1],
                    )
                tpv = tpb[:, 0 : 4 * 65].rearrange("p (m c) -> p m c", c=DV + 1)
                rcp = sbw.tile([128, 4], f32, tag="rcp")
                nc.vector.tensor_scalar_add(
                    rcp.rearrange("p (m c) -> p m c", c=1), tpv[:, :, DV : DV + 1], 1e-10
                )
                nc.vector.reciprocal(rcp[:], rcp[:])
                ot = sbw.tile([128, 4 * DV], f32, tag="ot")
                otv = ot.rearrange("p (m c) -> p m c", c=DV)
                nc.vector.tensor_mul(
                    otv,
                    tpv[:, :, 0:DV],
                    rcp.rearrange("p (m c) -> p m c", c=1).broadcast_to([128, 4, DV]),
                )
                nc.gpsimd.dma_start(
                    out=out_d[bass.ds(sc * SC, SC), :].rearrange(
                        "(m p) v -> p m v", p=128
                    ),
                    in_=otv,
                )

    nc.compile()
    return nc


def _in_maps(inputs):
    import ml_dtypes

    np_of = {"bfloat16": ml_dtypes.bfloat16, "float16": np.float16}
    qk_np = np_of.get(CFG["qk_dtype"], np.float32)
    v_np = np_of.get(CFG["v_dtype"], np.float32)
    q = np.asarray(inputs["q"], dtype=np.float32)
    k = np.asarray(inputs["k"], dtype=np.float32)
    v = np.asarray(inputs["v"], dtype=np.float32)
    pad = np.asarray(inputs["pad_masks"], dtype=np.float32)
    Wq = np.asarray(inputs["Wq"], dtype=np.float32)
    Wk = np.asarray(inputs["Wk"], dtype=np.float32)
    Wv = np.asarray(inputs["Wv"], dtype=np.float32)
    bq = np.asarray(inputs["bq"], dtype=np.float32)
    bk = np.asarray(inputs["bk"], dtype=np.float32)
    bv = np.asarray(inputs["bv"], dtype=np.float32)

    # block-diagonal q/k weights: wqk[p, 128cc + m] over 64-deep D-chunks
    wqk_p = np.zeros((128, 8 * 128), np.float32)
    for cc in range(8):
        wqk_p[0:64, 128 * cc : 128 * cc + 64] = Wq[64 * cc : 64 * cc + 64, :]
        wqk_p[64:128, 128 * cc + 64 : 128 * cc + 128] = Wk[
            64 * cc : 64 * cc + 64, :
        ]
    wqk_p = wqk_p.astype(qk_np)
    # wv packed [p, (c, m)]
    wv_p = np.ascontiguousarray(
        Wv.reshape(4, 128, 64).transpose(1, 0, 2).reshape(128, 256)
    ).astype(v_np)
    bias_qk = np.ascontiguousarray(np.concatenate([bq, bk]).reshape(128, 1))

    maps = []
    for b in range(B):
        qkcat = np.concatenate(
            [q[b].T.reshape(8, 64, S), k[b].T.reshape(8, 64, S)], axis=1
        )  # [8, 128, 2048]
        maps.append(
            {
                "qkT": np.ascontiguousarray(
                    qkcat.reshape(8, 128, 2, 1024)
                    .transpose(0, 2, 1, 3)
                    .astype(qk_np)
                ),
                "vT": np.ascontiguousarray(
                    v[b].T.reshape(4, 128, 2, 1024)
                    .transpose(0, 2, 1, 3)
                    .astype(v_np)
                ),
                "wqk": wqk_p,
                "wv": wv_p,
                "bias_qk": bias_qk,
                "bvrow": np.ascontiguousarray(bv.reshape(1, DV)),
                "padT": np.ascontiguousarray(pad[b, 0].reshape(NT, 128).T),
            }
        )
    return maps


def kernel(**inputs) -> np.ndarray:
    global _prog
    if _prog is None:
        _prog = _build_program()
    from concourse.bass_utils import run_bass_kernel_spmd

    res = run_bass_kernel_spmd(
        _prog, _in_maps(inputs), core_ids=list(range(NCORES)), trace=CFG["trace"]
    )
    kernel.last_result = res
    return np.stack([res.results[i]["out"] for i in range(NCORES)], axis=0)
